# revision 14
# baseline (speedup 1.0000x reference)
"""GCN regressor (3-layer GraphConv + mean-pool + linear head) on 8 Trainium2 cores.

Sharding: nodes (and their incident edges, bucketed by dst) are partitioned
across the 8 cores; each core owns 49 windows of 128 contiguous node ids
(node space padded 50000 -> 50176 = 8*49*128).  Edges live on the core that
owns their dst.

Host-side prep computes graph-structure quantities that involve no learned
weights: degrees, the deg^-0.5 norms, and the layer-0 aggregation (the
initial node feature is the in-degree, so layer 0's segment-sum input is a
pure structure scalar).  The device runs everything that touches the
network weights: h1 = relu(a0*W0row + b0), the two 64-dim GraphConv layers
(gather + one-hot-matmul segment-sum + dense transform), mean-pool, head.

Device pipeline per core:
  build tab1 = ns*relu(a0n x W0 + b0) -> chunked AllGather (a: groups 0-3,
  b: groups 4-6; AG_a overlaps the tail of the producing pass)
  -> layer pass (W1) -> chunked AllGather -> layer pass (W2)
  -> fused mean-pool -> AllGather(64x64 pooled partials) -> head -> y[64,1]

Layer pass details, per dst window (processed in groups of 7):
  - two dma_gathers (SWDGE, rotating queues) pull this window's msg rows
    h_tab[src] from the two replicated table segments (256B rows, int16
    indices); per-window granularity lets the PE start each window as soon
    as its rows land instead of waiting for a whole group drain
  - ~19 matmuls accumulate agg^T = sum msg^T (x) onehot into PSUM
    [64, 128]; the one-hot blocks are HOST-precomputed fp8 streamed in by
    DMA (frees the Vector engine)
  - Scalar engine casts PSUM -> bf16 into an augmented 65-row lhsT
    (row 64 = 1/nd, DMA-loaded per group); the transform matmul against
    [W; b] then adds the bias for free; one tensor_scalar does
    relu * (ns*nd) into the group table tile, written back per group.
"""

import os
import sys

if "/opt/trn_rl_repo" not in sys.path:
    sys.path.insert(0, "/opt/trn_rl_repo")

import numpy as np
import ml_dtypes

import concourse.bass as bass
import concourse.bacc as bacc
import concourse.tile as tile
from concourse import mybir
from concourse.bass_utils import run_bass_kernel_spmd

BF16 = ml_dtypes.bfloat16
FP8 = ml_dtypes.float8_e4m3
F32 = np.float32

NC = 8          # cores
P = 128         # partitions / window size
D = 64          # hidden dim
DPAD = 128      # padded table row (bf16) -> 256B rows for dma_gather
WPC = 49        # dst windows per core
NPC = WPC * P   # nodes per core = 6272
NP = NC * NPC   # padded node count = 50176
NG = 64         # graphs
N_NODES = 50000
G = 7           # windows per gather group
NGRP = WPC // G  # 7 gather groups per core

# src-segment split for chunked AllGathers (int16 gather indices require
# <= 4095 rows per core per segment: 8*4096 = 32768 > int16 max)
WA = 28          # segment-a windows per core (groups 0-3)
WB = WPC - WA    # segment-b windows per core (groups 4-6)
RA = WA * P      # 3584 rows/core, idx < 28672
RB = WB * P      # 2688 rows/core, idx < 21504

LAST_RESULTS = None  # test harness can read exec_time_ns etc. from here

_PROGRAM_CACHE = {}

STAGE = os.environ.get("KSTAGE", "full")  # tab1 | l1 | full


def _wrap16(flat, reps=8):
    """int array [n] -> int16 [16*reps, n//16] with element i at [i%16, i//16],
    replicated `reps` times down the partitions (Q7 core convention)."""
    n = flat.shape[0]
    a = flat.astype(np.int16).reshape(n // 16, 16).T
    return np.tile(a, (reps, 1))


def _build_program(M_A, M_B):
    M = M_A + M_B
    nc = bacc.Bacc("TRN2", target_bir_lowering=False, debug=False, num_devices=NC, num_swdge_queues=4)
    dt = mybir.dt

    # ---- I/O ----
    inp = {}

    def ein(name, shape, dtype):
        inp[name] = nc.dram_tensor(name, shape, dtype, kind="ExternalInput")
        return inp[name]

    ohd = ein("ohd", [P, WPC * M * P], dt.float8e4)     # edge one-hot blocks
    ohgd = ein("ohgd", [P, WPC * D], dt.float8e4)       # node->graph one-hot
    ga = ein("ga", [P, WPC * M_A * 8], dt.int16)
    gb = ein("gb", [P, WPC * M_B * 8], dt.int16)
    a0n = ein("a0n", [P, WPC], dt.float32)              # agg0 * nd
    ns_c = ein("ns_c", [P, WPC], dt.float32)            # outdeg^-0.5
    nd_c = ein("nd_c", [P, WPC], dt.float32)            # indeg^-0.5
    sn_c = ein("sn_c", [P, WPC], dt.float32)            # ns * nd
    rnd_d = ein("rnd_d", [1, WPC * P], dt.bfloat16)     # 1/nd row (aug bias)
    w0r = ein("w0r", [P, D], dt.bfloat16)
    b0r = ein("b0r", [P, D], dt.bfloat16)
    w1a = ein("w1a", [D + 1, D], dt.bfloat16)           # [W1; b1]
    w2a = ein("w2a", [D + 1, D], dt.bfloat16)           # [W2; b2]
    wr = ein("wr", [D, 1], dt.bfloat16)
    invc = ein("invc", [NG, 1], dt.float32)             # 1/max(counts,1)
    brc = ein("brc", [NG, 1], dt.float32)

    y = nc.dram_tensor("y", [NG, 1], dt.float32, kind="ExternalOutput")

    # ---- internal DRAM: per-layer tables, split into a/b segments ----
    tabLa = [nc.dram_tensor(f"tabLa{i}", [RA, DPAD], dt.bfloat16) for i in range(2)]
    tabLb = [nc.dram_tensor(f"tabLb{i}", [RB, DPAD], dt.bfloat16) for i in range(2)]
    tabFa = [
        nc.dram_tensor(f"tabFa{i}", [NC * RA, DPAD], dt.bfloat16, addr_space="Shared")
        for i in range(2)
    ]
    tabFb = [
        nc.dram_tensor(f"tabFb{i}", [NC * RB, DPAD], dt.bfloat16, addr_space="Shared")
        for i in range(2)
    ]
    headL = nc.dram_tensor("headL", [D, D], dt.float32)
    headF = nc.dram_tensor("headF", [NC * D, D], dt.float32, addr_space="Shared")

    rg = [list(range(NC))]
    OP = mybir.AluOpType
    AF = mybir.ActivationFunctionType

    with tile.TileContext(nc) as tc:
        with (
            tc.tile_pool(name="const", bufs=1) as cst,
            tc.tile_pool(name="sb", bufs=3) as sb,
            tc.tile_pool(name="grp", bufs=2) as grpp,
            tc.tile_pool(name="msgp", bufs=2) as msgp,
            tc.tile_pool(name="ohp", bufs=3) as ohp,
            tc.tile_pool(name="atp", bufs=3) as atp,
            tc.tile_pool(name="ps", bufs=2, space="PSUM") as ps,
            tc.tile_pool(name="ps2", bufs=2, space="PSUM") as ps2,
            tc.tile_pool(name="pshold", bufs=1, space="PSUM") as pshold,
        ):
            # ---- load constants / per-core arrays into SBUF ----
            def load(t):
                tt = cst.tile(list(t.shape), t.dtype, tag=f"ld_{t.name}")
                nc.sync.dma_start(out=tt[:], in_=t[:])
                return tt

            ga_t = load(ga)
            gb_t = load(gb)
            a0n_t = load(a0n)
            ns_t = load(ns_c)
            nd_t = load(nd_c)
            sn_t = load(sn_c)
            w0r_t = load(w0r)
            b0r_t = load(b0r)
            w1a_t = load(w1a)
            w2a_t = load(w2a)
            wr_t = load(wr)
            invc_t = load(invc)
            brc_t = load(brc)
            ohg_t = load(ohgd)

            def write_group(li, g2, grp):
                """DMA a group's [P, G*D] bf16 tile into the right table segment."""
                if g2 < 4:
                    tab, base = (tabLa[li], g2 * G * P)
                else:
                    tab, base = (tabLb[li], (g2 - 4) * G * P)
                nc.sync.dma_start(
                    out=tab[base : base + G * P, 0:D].rearrange(
                        "(g p) d -> p g d", g=G
                    ),
                    in_=grp[:].rearrange("p (g d) -> p g d", g=G),
                )

            def ag_a(li):
                nc.gpsimd.collective_compute(
                    "AllGather", OP.bypass, replica_groups=rg,
                    ins=[tabLa[li][:]], outs=[tabFa[li][:]],
                )

            def ag_b(li):
                nc.gpsimd.collective_compute(
                    "AllGather", OP.bypass, replica_groups=rg,
                    ins=[tabLb[li][:]], outs=[tabFb[li][:]],
                )

            # ---- tab1 = ns * relu(a0n * W0row + b0), chunked AG ----
            for g2 in range(NGRP):
                grp = grpp.tile([P, G * D], dt.bfloat16, tag="grp1")
                for g in range(G):
                    w = g2 * G + g
                    t0 = sb.tile([P, D], dt.bfloat16, tag="t0")
                    nc.vector.tensor_scalar(
                        out=t0[:], in0=w0r_t[:], scalar1=a0n_t[:, w : w + 1],
                        scalar2=None, op0=OP.mult,
                    )
                    nc.vector.tensor_tensor(
                        out=t0[:], in0=t0[:], in1=b0r_t[:], op=OP.add
                    )
                    nc.vector.tensor_scalar(
                        out=grp[:, g * D : (g + 1) * D], in0=t0[:],
                        scalar1=0.0, op0=OP.max,
                        scalar2=ns_t[:, w : w + 1], op1=OP.mult,
                    )
                write_group(0, g2, grp)
                if g2 == 3:
                    ag_a(0)
            ag_b(0)

            def dbg_from_tab(tab):
                tt = sb.tile([NG, 1], dt.bfloat16, tag="dbgt")
                nc.sync.dma_start(out=tt[:], in_=tab[0:NG, 0:1])
                yv = sb.tile([NG, 1], dt.float32, tag="dbg")
                nc.vector.tensor_copy(out=yv[:], in_=tt[:])
                nc.sync.dma_start(out=y[:], in_=yv[:])

            pool_ps = pshold.tile([D, D], dt.float32)

            # ---- layer pass: per-window gathers + one-hot matmul agg ----
            def layer_pass(li, wa_t, last):
                qn = 0
                for g2 in range(NGRP):
                    oh = ohp.tile([P, G * M * P], dt.float8e4, tag="oh")
                    nc.sync.dma_start(
                        out=oh[:], in_=ohd[:, g2 * G * M * P : (g2 + 1) * G * M * P]
                    )
                    msg = msgp.tile([P, G * M * P], dt.bfloat16, tag="msg")
                    msg3 = msg[:].rearrange("p (c d) -> p c d", d=P)

                    # aug lhsT tiles: row D = 1/nd for the group's windows
                    aT = atp.tile([D + 1, G * P], dt.bfloat16, tag="aT")
                    nc.sync.dma_start(
                        out=aT[D : D + 1, :],
                        in_=rnd_d[0:1, g2 * G * P : (g2 + 1) * G * P],
                    )
                    if not last:
                        grp = grpp.tile([P, G * D], dt.bfloat16, tag="grp")

                    # window-pair gathers: larger contiguous descriptor runs
                    # per queue, while PE only waits on a 2-window drain
                    for p in range(0, G, 2):
                        wn = min(2, G - p)
                        wg = g2 * G + p
                        nc.gpsimd.dma_gather(
                            out_ap=msg3[:, p * M_A : (p + wn) * M_A, :],
                            in_ap=tabFa[li][:],
                            idxs_ap=ga_t[:, wg * M_A * 8 : (wg + wn) * M_A * 8],
                            num_idxs=wn * M_A * P, num_idxs_reg=wn * M_A * P,
                            elem_size=DPAD, single_packet=False,
                            queue_num=qn % 4,
                        )
                        qn += 1
                        nc.gpsimd.dma_gather(
                            out_ap=msg3[
                                :, G * M_A + p * M_B : G * M_A + (p + wn) * M_B, :
                            ],
                            in_ap=tabFb[li][:],
                            idxs_ap=gb_t[:, wg * M_B * 8 : (wg + wn) * M_B * 8],
                            num_idxs=wn * M_B * P, num_idxs_reg=wn * M_B * P,
                            elem_size=DPAD, single_packet=False,
                            queue_num=qn % 4,
                        )
                        qn += 1

                    for g in range(G):
                        w = g2 * G + g
                        pa = ps.tile([D, P], dt.float32, tag="pa")
                        k = 0
                        for base, Mh in ((g * M_A, M_A), (G * M_A + g * M_B, M_B)):
                            for kk in range(Mh):
                                chunk = base + kk
                                nc.tensor.matmul(
                                    out=pa[:],
                                    lhsT=msg[:, chunk * P : chunk * P + D],
                                    rhs=oh[:, chunk * P : (chunk + 1) * P],
                                    start=(k == 0), stop=(k == M - 1),
                                )
                                k += 1
                        # cast agg^T into aug lhsT rows 0..D-1 (scalar engine)
                        nc.scalar.activation(
                            out=aT[0:D, g * P : (g + 1) * P], in_=pa[:], func=AF.Copy
                        )
                        ph = ps2.tile([P, D], dt.float32, tag="ph")
                        nc.tensor.matmul(
                            out=ph[:], lhsT=aT[:, g * P : (g + 1) * P], rhs=wa_t[:],
                            start=True, stop=True,
                        )
                        if not last:
                            # table tile = sn * relu(agg @ W + rnd*b)
                            nc.vector.tensor_scalar(
                                out=grp[:, g * D : (g + 1) * D], in0=ph[:],
                                scalar1=0.0, op0=OP.max,
                                scalar2=sn_t[:, w : w + 1], op1=OP.mult,
                            )
                        else:
                            h = sb.tile([P, D], dt.bfloat16, tag="h")
                            nc.vector.tensor_scalar(
                                out=h[:], in0=ph[:],
                                scalar1=0.0, op0=OP.max,
                                scalar2=nd_t[:, w : w + 1], op1=OP.mult,
                            )
                            nc.tensor.matmul(
                                out=pool_ps[:], lhsT=h[:],
                                rhs=ohg_t[:, w * D : (w + 1) * D],
                                start=(w == 0), stop=(w == WPC - 1),
                                skip_group_check=True,
                            )
                    if not last:
                        write_group(li + 1, g2, grp)
                        if g2 == 3:
                            ag_a(li + 1)
                if not last:
                    ag_b(li + 1)

            do_head = False
            if STAGE == "tab1":
                dbg_from_tab(tabFa[0])
            else:
                layer_pass(0, w1a_t, last=False)
                if STAGE == "l1":
                    dbg_from_tab(tabFa[1])
                else:
                    layer_pass(1, w2a_t, last=True)
                    do_head = True

            # ---- head ----
            if do_head:
                hg_sb = sb.tile([D, D], dt.float32)
                nc.vector.tensor_copy(out=hg_sb[:], in_=pool_ps[:])
                nc.sync.dma_start(out=headL[:], in_=hg_sb[:])
                nc.gpsimd.collective_compute(
                    "AllGather", OP.bypass, replica_groups=rg,
                    ins=[headL[:]], outs=[headF[:]],
                )
                hg_acc = sb.tile([D, D], dt.float32)
                nc.sync.dma_start(out=hg_acc[:], in_=headF[0:D, :])
                for k in range(1, NC):
                    tk = sb.tile([D, D], dt.float32, tag="tk")
                    nc.sync.dma_start(out=tk[:], in_=headF[k * D : (k + 1) * D, :])
                    nc.vector.tensor_tensor(
                        out=hg_acc[:], in0=hg_acc[:], in1=tk[:], op=OP.add
                    )
                hg_bf = sb.tile([D, D], dt.bfloat16)
                nc.vector.tensor_copy(out=hg_bf[:], in_=hg_acc[:])
                po = ps.tile([NG, 1], dt.float32, tag="po")
                nc.tensor.matmul(out=po[:], lhsT=hg_bf[:], rhs=wr_t[:], start=True, stop=True)
                yv = sb.tile([NG, 1], dt.float32)
                nc.vector.tensor_scalar(
                    out=yv[:], in0=po[:], scalar1=invc_t[:, 0:1], op0=OP.mult,
                    scalar2=brc_t[:, 0:1], op1=OP.add,
                )
                nc.sync.dma_start(out=y[:], in_=yv[:])

    nc.finalize()
    return nc


def _prep_inputs(src, dst, node2graph, W0, b0, W1, b1, W2, b2, Wr, br):
    src = np.asarray(src, dtype=np.int64)
    dst = np.asarray(dst, dtype=np.int64)
    n2g_in = np.asarray(node2graph, dtype=np.int64)
    E = src.shape[0]

    # ---- host graph structure: degrees, norms, layer-0 aggregation ----
    in_deg = np.bincount(dst, minlength=NP).astype(F32)
    out_deg = np.bincount(src, minlength=NP).astype(F32)
    ns = np.maximum(out_deg, 1.0) ** -0.5
    nd = np.maximum(in_deg, 1.0) ** -0.5
    s0 = in_deg * ns                       # (h*norm_src) with h = in_deg
    agg0 = np.bincount(dst, weights=s0[src], minlength=NP).astype(F32)
    a0n_full = agg0 * nd                   # agg0 * norm_dst
    sn = ns * nd
    rnd = np.maximum(in_deg, 1.0) ** 0.5   # 1/nd

    def per_core_cols(v):
        # [NP] -> [NC, P, WPC]  (node (c,w,p) at [c, p, w])
        return v.reshape(NC, WPC, P).transpose(0, 2, 1).copy()

    a0n_all = per_core_cols(a0n_full)
    ns_all = per_core_cols(ns)
    nd_all = per_core_cols(nd)
    sn_all = per_core_cols(sn)
    rnd_all = rnd.reshape(NC, 1, NPC).astype(BF16)   # [c, 1, WPC*P]

    # ---- dst bucketing: (global dst window, src segment a/b) ----
    win = dst >> 7  # 0..391
    off = dst & 127
    src_core = src // NPC
    src_loc = src % NPC
    in_b = (src_loc >= RA).astype(np.int64)
    key = win * 2 + in_b
    order = np.argsort(key, kind="stable")
    key_s = key[order]
    counts = np.bincount(key_s, minlength=NC * WPC * 2)
    n_a = counts[0::2]
    n_b = counts[1::2]
    M_A = int(np.ceil(n_a.max() / P))
    M_B = int(np.ceil(n_b.max() / P))
    M = M_A + M_B

    starts = np.zeros(NC * WPC * 2, dtype=np.int64)
    starts[1:] = np.cumsum(counts)[:-1]
    rank = np.arange(E) - starts[key_s]  # rank within (window, segment)
    win_s = win[order]
    b_s = key_s & 1
    off_s = off[order]
    idx_s = np.where(
        b_s == 1,
        src_core[order] * RB + (src_loc[order] - RA),
        src_core[order] * RA + src_loc[order],
    )

    # edge slot within window: a -> rank, b -> M_A*128 + rank
    slot = np.where(b_s == 1, M_A * P + rank, rank)
    row = (slot & 127).astype(np.int64)   # partition (slot within block)
    wl = win_s % WPC
    g = wl % G
    g2 = wl // G
    blk = slot >> 7                        # block within window (0..M-1)
    core = win_s // WPC

    # group chunk layout: [a-spans of the 7 windows | b-spans]
    gchunk = np.where(
        b_s == 1, G * M_A + g * M_B + (blk - M_A), g * M_A + blk
    )
    # one-hot blocks, fp8: [c, p(slot%128), ((g2*G*M + gchunk)*128 + dstoff)]
    oh_all = np.zeros((NC, P, WPC * M * P), dtype=FP8)
    oh_all[core, row, (g2 * (G * M) + gchunk) * P + off_s] = FP8(1.0)

    # gather index lists, window-major
    ga_flat = np.zeros((NC, WPC, M_A * P), dtype=np.int64)
    gb_flat = np.zeros((NC, WPC, M_B * P), dtype=np.int64)
    am = b_s == 0
    ga_flat[core[am], wl[am], slot[am]] = idx_s[am]
    bm = b_s == 1
    gb_flat[core[bm], wl[bm], slot[bm] - M_A * P] = idx_s[bm]

    ga_w = np.zeros((NC, P, WPC * M_A * 8), dtype=np.int16)
    gb_w = np.zeros((NC, P, WPC * M_B * 8), dtype=np.int16)
    for c in range(NC):
        for w in range(WPC):
            ga_w[c][:, w * M_A * 8 : (w + 1) * M_A * 8] = _wrap16(ga_flat[c, w])
            gb_w[c][:, w * M_B * 8 : (w + 1) * M_B * 8] = _wrap16(gb_flat[c, w])

    # ---- node->graph one-hot (fp8) and per-graph inverse counts ----
    n2g_pad = np.full(NP, NG, dtype=np.int64)   # pad windows -> no graph
    n2g_pad[:N_NODES] = n2g_in
    gids = np.arange(D)
    ohg_all = (
        n2g_pad.reshape(NC, WPC, P).transpose(0, 2, 1)[:, :, :, None] == gids
    ).astype(FP8)                                # [c, p, w, 64]
    ohg_all = ohg_all.reshape(NC, P, WPC * D)
    g_counts = np.bincount(n2g_in, minlength=NG).astype(F32)
    invc_np = (1.0 / np.maximum(g_counts, 1.0)).reshape(NG, 1)

    W0 = np.asarray(W0, F32)
    common = {
        "w0r": np.tile(W0.reshape(1, D), (P, 1)).astype(BF16),
        "b0r": np.tile(np.asarray(b0, F32).reshape(1, D), (P, 1)).astype(BF16),
        "w1a": np.concatenate(
            [np.asarray(W1, F32), np.asarray(b1, F32).reshape(1, D)], axis=0
        ).astype(BF16),
        "w2a": np.concatenate(
            [np.asarray(W2, F32), np.asarray(b2, F32).reshape(1, D)], axis=0
        ).astype(BF16),
        "wr": np.asarray(Wr, F32).reshape(D, 1).astype(BF16),
        "invc": invc_np,
        "brc": np.full((NG, 1), float(np.asarray(br).reshape(-1)[0]), dtype=F32),
    }
    in_maps = []
    for c in range(NC):
        m = dict(common)
        m["ohd"] = oh_all[c]
        m["ohgd"] = np.ascontiguousarray(ohg_all[c])
        m["ga"] = ga_w[c]
        m["gb"] = gb_w[c]
        m["a0n"] = np.ascontiguousarray(a0n_all[c])
        m["ns_c"] = np.ascontiguousarray(ns_all[c])
        m["nd_c"] = np.ascontiguousarray(nd_all[c])
        m["sn_c"] = np.ascontiguousarray(sn_all[c])
        m["rnd_d"] = np.ascontiguousarray(rnd_all[c])
        in_maps.append(m)
    return (M_A, M_B), in_maps


def kernel(src, dst, node2graph, W0, b0, W1, b1, W2, b2, Wr, br):
    global LAST_RESULTS
    (M_A, M_B), in_maps = _prep_inputs(
        src, dst, node2graph, W0, b0, W1, b1, W2, b2, Wr, br
    )
    key = (M_A, M_B)
    if key not in _PROGRAM_CACHE:
        _PROGRAM_CACHE[key] = _build_program(M_A, M_B)
    nc = _PROGRAM_CACHE[key]
    res = run_bass_kernel_spmd(nc, in_maps, core_ids=list(range(NC)))
    LAST_RESULTS = res
    return np.asarray(res.results[0]["y"], dtype=np.float32)


# revision 16
# speedup vs baseline: 1.0833x; 1.0833x over previous
"""GCN regressor (3-layer GraphConv + mean-pool + linear head) on 8 Trainium2 cores.

Sharding: nodes (and their incident edges, bucketed by dst) are partitioned
across the 8 cores; each core owns 49 windows of 128 contiguous node ids
(node space padded 50000 -> 50176 = 8*49*128).  Edges live on the core that
owns their dst.

Host-side prep computes graph-structure quantities that involve no learned
weights: degrees, the deg^-0.5 norms, and the layer-0 aggregation (the
initial node feature is the in-degree, so layer 0's segment-sum input is a
pure structure scalar).  The device runs everything that touches the
network weights: h1 = relu(a0*W0row + b0), the two 64-dim GraphConv layers
(gather + one-hot-matmul segment-sum + dense transform), mean-pool, head.

Device pipeline per core:
  build tab1 = ns*relu(a0n x W0 + b0) -> chunked AllGather (a: groups 0-3,
  b: groups 4-6; AG_a overlaps the tail of the producing pass)
  -> layer pass (W1) -> chunked AllGather -> layer pass (W2)
  -> fused mean-pool -> AllGather(64x64 pooled partials) -> head -> y[64,1]

Layer pass details, per dst window (processed in groups of 7):
  - two dma_gathers (SWDGE, rotating queues) pull this window's msg rows
    h_tab[src] from the two replicated table segments (256B rows, int16
    indices); per-window granularity lets the PE start each window as soon
    as its rows land instead of waiting for a whole group drain
  - ~19 matmuls accumulate agg^T = sum msg^T (x) onehot into PSUM
    [64, 128]; the one-hot blocks are HOST-precomputed fp8 streamed in by
    DMA (frees the Vector engine)
  - Scalar engine casts PSUM -> bf16 into an augmented 65-row lhsT
    (row 64 = 1/nd, DMA-loaded per group); the transform matmul against
    [W; b] then adds the bias for free; one tensor_scalar does
    relu * (ns*nd) into the group table tile, written back per group.
"""

import os
import sys

if "/opt/trn_rl_repo" not in sys.path:
    sys.path.insert(0, "/opt/trn_rl_repo")

import numpy as np
import ml_dtypes

import concourse.bass as bass
import concourse.bacc as bacc
import concourse.tile as tile
from concourse import mybir
from concourse.bass_utils import run_bass_kernel_spmd

BF16 = ml_dtypes.bfloat16
FP8 = ml_dtypes.float8_e4m3
F32 = np.float32

NC = 8          # cores
P = 128         # partitions / window size
D = 64          # hidden dim
DPAD = 128      # padded table row (bf16) -> 256B rows for dma_gather
WPC = 49        # dst windows per core
NPC = WPC * P   # nodes per core = 6272
NP = NC * NPC   # padded node count = 50176
NG = 64         # graphs
N_NODES = 50000
G = 7           # windows per gather group
NGRP = WPC // G  # 7 gather groups per core

# src-segment split for chunked AllGathers (int16 gather indices require
# <= 4095 rows per core per segment: 8*4096 = 32768 > int16 max)
WA = 28          # segment-a windows per core (groups 0-3)
WB = WPC - WA    # segment-b windows per core (groups 4-6)
RA = WA * P      # 3584 rows/core, idx < 28672
RB = WB * P      # 2688 rows/core, idx < 21504

LAST_RESULTS = None  # test harness can read exec_time_ns etc. from here

_PROGRAM_CACHE = {}

STAGE = os.environ.get("KSTAGE", "full")  # tab1 | l1 | full


def _wrap16(flat, reps=8):
    """int array [n] -> int16 [16*reps, n//16] with element i at [i%16, i//16],
    replicated `reps` times down the partitions (Q7 core convention)."""
    n = flat.shape[0]
    a = flat.astype(np.int16).reshape(n // 16, 16).T
    return np.tile(a, (reps, 1))


def _build_program(M_A, M_B):
    M = M_A + M_B
    nc = bacc.Bacc("TRN2", target_bir_lowering=False, debug=False, num_devices=NC, num_swdge_queues=4)
    dt = mybir.dt

    # ---- I/O ----
    inp = {}

    def ein(name, shape, dtype):
        inp[name] = nc.dram_tensor(name, shape, dtype, kind="ExternalInput")
        return inp[name]

    ohd = ein("ohd", [P, WPC * M * P], dt.float8e4)     # edge one-hot blocks
    ohgd = ein("ohgd", [P, WPC * D], dt.float8e4)       # node->graph one-hot
    ga = ein("ga", [P, WPC * M_A * 8], dt.int16)
    gb = ein("gb", [P, WPC * M_B * 8], dt.int16)
    a0n = ein("a0n", [P, WPC], dt.float32)              # agg0 * nd
    ns_c = ein("ns_c", [P, WPC], dt.float32)            # outdeg^-0.5
    nd_c = ein("nd_c", [P, WPC], dt.float32)            # indeg^-0.5
    sn_c = ein("sn_c", [P, WPC], dt.float32)            # ns * nd
    rnd_d = ein("rnd_d", [1, WPC * P], dt.bfloat16)     # 1/nd row (aug bias)
    w0r = ein("w0r", [P, D], dt.bfloat16)
    b0r = ein("b0r", [P, D], dt.bfloat16)
    w1a = ein("w1a", [D + 1, D], dt.bfloat16)           # [W1; b1]
    w2a = ein("w2a", [D + 1, D], dt.bfloat16)           # [W2; b2]
    wr = ein("wr", [D, 1], dt.bfloat16)
    invc = ein("invc", [NG, 1], dt.float32)             # 1/max(counts,1)
    brc = ein("brc", [NG, 1], dt.float32)

    y = nc.dram_tensor("y", [NG, 1], dt.float32, kind="ExternalOutput")

    # ---- internal DRAM: per-layer tables, split into a/b segments ----
    tabLa = [nc.dram_tensor(f"tabLa{i}", [RA, DPAD], dt.bfloat16) for i in range(2)]
    tabLb = [nc.dram_tensor(f"tabLb{i}", [RB, DPAD], dt.bfloat16) for i in range(2)]
    tabFa = [
        nc.dram_tensor(f"tabFa{i}", [NC * RA, DPAD], dt.bfloat16, addr_space="Shared")
        for i in range(2)
    ]
    tabFb = [
        nc.dram_tensor(f"tabFb{i}", [NC * RB, DPAD], dt.bfloat16, addr_space="Shared")
        for i in range(2)
    ]
    headL = nc.dram_tensor("headL", [D, D], dt.float32)
    headF = nc.dram_tensor("headF", [NC * D, D], dt.float32, addr_space="Shared")

    rg = [list(range(NC))]
    OP = mybir.AluOpType
    AF = mybir.ActivationFunctionType

    with tile.TileContext(nc) as tc:
        with (
            tc.tile_pool(name="const", bufs=1) as cst,
            tc.tile_pool(name="sb", bufs=3) as sb,
            tc.tile_pool(name="grp", bufs=2) as grpp,
            tc.tile_pool(name="msgp", bufs=2) as msgp,
            tc.tile_pool(name="ohp", bufs=3) as ohp,
            tc.tile_pool(name="atp", bufs=3) as atp,
            tc.tile_pool(name="ps", bufs=2, space="PSUM") as ps,
            tc.tile_pool(name="ps2", bufs=2, space="PSUM") as ps2,
            tc.tile_pool(name="pshold", bufs=1, space="PSUM") as pshold,
        ):
            # ---- load constants / per-core arrays into SBUF ----
            def load(t):
                tt = cst.tile(list(t.shape), t.dtype, tag=f"ld_{t.name}")
                nc.sync.dma_start(out=tt[:], in_=t[:])
                return tt

            ga_t = load(ga)
            gb_t = load(gb)
            a0n_t = load(a0n)
            ns_t = load(ns_c)
            nd_t = load(nd_c)
            sn_t = load(sn_c)
            w0r_t = load(w0r)
            b0r_t = load(b0r)
            w1a_t = load(w1a)
            w2a_t = load(w2a)
            wr_t = load(wr)
            invc_t = load(invc)
            brc_t = load(brc)
            ohg_t = load(ohgd)

            def write_group(li, g2, grp):
                """DMA a group's [P, G*D] bf16 tile into the right table segment."""
                if g2 < 4:
                    tab, base = (tabLa[li], g2 * G * P)
                else:
                    tab, base = (tabLb[li], (g2 - 4) * G * P)
                nc.sync.dma_start(
                    out=tab[base : base + G * P, 0:D].rearrange(
                        "(g p) d -> p g d", g=G
                    ),
                    in_=grp[:].rearrange("p (g d) -> p g d", g=G),
                )

            def ag_a(li):
                nc.gpsimd.collective_compute(
                    "AllGather", OP.bypass, replica_groups=rg,
                    ins=[tabLa[li][:]], outs=[tabFa[li][:]],
                )

            def ag_b(li):
                nc.gpsimd.collective_compute(
                    "AllGather", OP.bypass, replica_groups=rg,
                    ins=[tabLb[li][:]], outs=[tabFb[li][:]],
                )

            # ---- tab1 = ns * relu(a0n * W0row + b0), chunked AG ----
            for g2 in range(NGRP):
                grp = grpp.tile([P, G * D], dt.bfloat16, tag="grp1")
                for g in range(G):
                    w = g2 * G + g
                    t0 = sb.tile([P, D], dt.bfloat16, tag="t0")
                    nc.vector.tensor_scalar(
                        out=t0[:], in0=w0r_t[:], scalar1=a0n_t[:, w : w + 1],
                        scalar2=None, op0=OP.mult,
                    )
                    nc.vector.tensor_tensor(
                        out=t0[:], in0=t0[:], in1=b0r_t[:], op=OP.add
                    )
                    nc.vector.tensor_scalar(
                        out=grp[:, g * D : (g + 1) * D], in0=t0[:],
                        scalar1=0.0, op0=OP.max,
                        scalar2=ns_t[:, w : w + 1], op1=OP.mult,
                    )
                write_group(0, g2, grp)
                if g2 == 3:
                    ag_a(0)
            ag_b(0)

            def dbg_from_tab(tab):
                tt = sb.tile([NG, 1], dt.bfloat16, tag="dbgt")
                nc.sync.dma_start(out=tt[:], in_=tab[0:NG, 0:1])
                yv = sb.tile([NG, 1], dt.float32, tag="dbg")
                nc.vector.tensor_copy(out=yv[:], in_=tt[:])
                nc.sync.dma_start(out=y[:], in_=yv[:])

            pool_ps = pshold.tile([D, D], dt.float32)

            # ---- layer pass: per-window gathers + one-hot matmul agg ----
            def layer_pass(li, wa_t, last):
                qn = 0
                for g2 in range(NGRP):
                    oh = ohp.tile([P, G * M * P], dt.float8e4, tag="oh")
                    nc.sync.dma_start(
                        out=oh[:], in_=ohd[:, g2 * G * M * P : (g2 + 1) * G * M * P]
                    )
                    msg = msgp.tile([P, G * M * P], dt.bfloat16, tag="msg")
                    msg3 = msg[:].rearrange("p (c d) -> p c d", d=P)

                    # aug lhsT tiles: row D = 1/nd for the group's windows
                    aT = atp.tile([D + 1, G * P], dt.bfloat16, tag="aT")
                    nc.sync.dma_start(
                        out=aT[D : D + 1, :],
                        in_=rnd_d[0:1, g2 * G * P : (g2 + 1) * G * P],
                    )
                    if not last:
                        grp = grpp.tile([P, G * D], dt.bfloat16, tag="grp")

                    for g in range(G):
                        w = g2 * G + g
                        wb = w * 8
                        # both gathers of a window share one queue so its
                        # completion sems fire together; queues rotate per
                        # window -> 4 windows drain concurrently
                        nc.gpsimd.dma_gather(
                            out_ap=msg3[:, g * M : g * M + M_A, :],
                            in_ap=tabFa[li][:],
                            idxs_ap=ga_t[:, wb * M_A : (wb + 8) * M_A],
                            num_idxs=M_A * P, num_idxs_reg=M_A * P,
                            elem_size=DPAD, single_packet=False,
                            queue_num=qn % 4,
                        )
                        nc.gpsimd.dma_gather(
                            out_ap=msg3[:, g * M + M_A : (g + 1) * M, :],
                            in_ap=tabFb[li][:],
                            idxs_ap=gb_t[:, wb * M_B : (wb + 8) * M_B],
                            num_idxs=M_B * P, num_idxs_reg=M_B * P,
                            elem_size=DPAD, single_packet=False,
                            queue_num=qn % 4,
                        )
                        qn += 1

                        pa = ps.tile([D, P], dt.float32, tag="pa")
                        for k in range(M):
                            chunk = g * M + k
                            nc.tensor.matmul(
                                out=pa[:],
                                lhsT=msg[:, chunk * P : chunk * P + D],
                                rhs=oh[:, chunk * P : (chunk + 1) * P],
                                start=(k == 0), stop=(k == M - 1),
                            )
                        # cast agg^T into aug lhsT rows 0..D-1 (scalar engine)
                        nc.scalar.activation(
                            out=aT[0:D, g * P : (g + 1) * P], in_=pa[:], func=AF.Copy
                        )
                        ph = ps2.tile([P, D], dt.float32, tag="ph")
                        nc.tensor.matmul(
                            out=ph[:], lhsT=aT[:, g * P : (g + 1) * P], rhs=wa_t[:],
                            start=True, stop=True,
                        )
                        if not last:
                            # table tile = sn * relu(agg @ W + rnd*b)
                            nc.vector.tensor_scalar(
                                out=grp[:, g * D : (g + 1) * D], in0=ph[:],
                                scalar1=0.0, op0=OP.max,
                                scalar2=sn_t[:, w : w + 1], op1=OP.mult,
                            )
                        else:
                            h = sb.tile([P, D], dt.bfloat16, tag="h")
                            nc.vector.tensor_scalar(
                                out=h[:], in0=ph[:],
                                scalar1=0.0, op0=OP.max,
                                scalar2=nd_t[:, w : w + 1], op1=OP.mult,
                            )
                            nc.tensor.matmul(
                                out=pool_ps[:], lhsT=h[:],
                                rhs=ohg_t[:, w * D : (w + 1) * D],
                                start=(w == 0), stop=(w == WPC - 1),
                                skip_group_check=True,
                            )
                    if not last:
                        write_group(li + 1, g2, grp)
                        if g2 == 3:
                            ag_a(li + 1)
                if not last:
                    ag_b(li + 1)

            do_head = False
            if STAGE == "tab1":
                dbg_from_tab(tabFa[0])
            else:
                layer_pass(0, w1a_t, last=False)
                if STAGE == "l1":
                    dbg_from_tab(tabFa[1])
                else:
                    layer_pass(1, w2a_t, last=True)
                    do_head = True

            # ---- head ----
            if do_head:
                hg_sb = sb.tile([D, D], dt.float32)
                nc.vector.tensor_copy(out=hg_sb[:], in_=pool_ps[:])
                nc.sync.dma_start(out=headL[:], in_=hg_sb[:])
                nc.gpsimd.collective_compute(
                    "AllGather", OP.bypass, replica_groups=rg,
                    ins=[headL[:]], outs=[headF[:]],
                )
                hg_acc = sb.tile([D, D], dt.float32)
                nc.sync.dma_start(out=hg_acc[:], in_=headF[0:D, :])
                for k in range(1, NC):
                    tk = sb.tile([D, D], dt.float32, tag="tk")
                    nc.sync.dma_start(out=tk[:], in_=headF[k * D : (k + 1) * D, :])
                    nc.vector.tensor_tensor(
                        out=hg_acc[:], in0=hg_acc[:], in1=tk[:], op=OP.add
                    )
                hg_bf = sb.tile([D, D], dt.bfloat16)
                nc.vector.tensor_copy(out=hg_bf[:], in_=hg_acc[:])
                po = ps.tile([NG, 1], dt.float32, tag="po")
                nc.tensor.matmul(out=po[:], lhsT=hg_bf[:], rhs=wr_t[:], start=True, stop=True)
                yv = sb.tile([NG, 1], dt.float32)
                nc.vector.tensor_scalar(
                    out=yv[:], in0=po[:], scalar1=invc_t[:, 0:1], op0=OP.mult,
                    scalar2=brc_t[:, 0:1], op1=OP.add,
                )
                nc.sync.dma_start(out=y[:], in_=yv[:])

    nc.finalize()
    return nc


def _prep_inputs(src, dst, node2graph, W0, b0, W1, b1, W2, b2, Wr, br):
    src = np.asarray(src, dtype=np.int64)
    dst = np.asarray(dst, dtype=np.int64)
    n2g_in = np.asarray(node2graph, dtype=np.int64)
    E = src.shape[0]

    # ---- host graph structure: degrees, norms, layer-0 aggregation ----
    in_deg = np.bincount(dst, minlength=NP).astype(F32)
    out_deg = np.bincount(src, minlength=NP).astype(F32)
    ns = np.maximum(out_deg, 1.0) ** -0.5
    nd = np.maximum(in_deg, 1.0) ** -0.5
    s0 = in_deg * ns                       # (h*norm_src) with h = in_deg
    agg0 = np.bincount(dst, weights=s0[src], minlength=NP).astype(F32)
    a0n_full = agg0 * nd                   # agg0 * norm_dst
    sn = ns * nd
    rnd = np.maximum(in_deg, 1.0) ** 0.5   # 1/nd

    def per_core_cols(v):
        # [NP] -> [NC, P, WPC]  (node (c,w,p) at [c, p, w])
        return v.reshape(NC, WPC, P).transpose(0, 2, 1).copy()

    a0n_all = per_core_cols(a0n_full)
    ns_all = per_core_cols(ns)
    nd_all = per_core_cols(nd)
    sn_all = per_core_cols(sn)
    rnd_all = rnd.reshape(NC, 1, NPC).astype(BF16)   # [c, 1, WPC*P]

    # ---- dst bucketing: (global dst window, src segment a/b) ----
    win = dst >> 7  # 0..391
    off = dst & 127
    src_core = src // NPC
    src_loc = src % NPC
    in_b = (src_loc >= RA).astype(np.int64)
    key = win * 2 + in_b
    order = np.argsort(key, kind="stable")
    key_s = key[order]
    counts = np.bincount(key_s, minlength=NC * WPC * 2)
    n_a = counts[0::2]
    n_b = counts[1::2]
    M_A = int(np.ceil(n_a.max() / P))
    M_B = int(np.ceil(n_b.max() / P))
    M = M_A + M_B

    starts = np.zeros(NC * WPC * 2, dtype=np.int64)
    starts[1:] = np.cumsum(counts)[:-1]
    rank = np.arange(E) - starts[key_s]  # rank within (window, segment)
    win_s = win[order]
    b_s = key_s & 1
    off_s = off[order]
    idx_s = np.where(
        b_s == 1,
        src_core[order] * RB + (src_loc[order] - RA),
        src_core[order] * RA + src_loc[order],
    )

    # edge slot within window: a -> rank, b -> M_A*128 + rank
    slot = np.where(b_s == 1, M_A * P + rank, rank)
    row = (slot & 127).astype(np.int64)   # partition (slot within block)
    wl = win_s % WPC
    g = wl % G
    g2 = wl // G
    blk = slot >> 7                        # chunk within window (0..M-1)
    core = win_s // WPC

    # one-hot blocks, fp8: [c, p(slot%128), ((w*M + blk)*128 + dstoff)]
    oh_all = np.zeros((NC, P, WPC * M * P), dtype=FP8)
    oh_all[core, row, ((g2 * G + g) * M + blk) * P + off_s] = FP8(1.0)

    # gather index lists, window-major
    ga_flat = np.zeros((NC, WPC, M_A * P), dtype=np.int64)
    gb_flat = np.zeros((NC, WPC, M_B * P), dtype=np.int64)
    am = b_s == 0
    ga_flat[core[am], wl[am], slot[am]] = idx_s[am]
    bm = b_s == 1
    gb_flat[core[bm], wl[bm], slot[bm] - M_A * P] = idx_s[bm]

    ga_w = np.zeros((NC, P, WPC * M_A * 8), dtype=np.int16)
    gb_w = np.zeros((NC, P, WPC * M_B * 8), dtype=np.int16)
    for c in range(NC):
        for w in range(WPC):
            ga_w[c][:, w * M_A * 8 : (w + 1) * M_A * 8] = _wrap16(ga_flat[c, w])
            gb_w[c][:, w * M_B * 8 : (w + 1) * M_B * 8] = _wrap16(gb_flat[c, w])

    # ---- node->graph one-hot (fp8) and per-graph inverse counts ----
    n2g_pad = np.full(NP, NG, dtype=np.int64)   # pad windows -> no graph
    n2g_pad[:N_NODES] = n2g_in
    gids = np.arange(D)
    ohg_all = (
        n2g_pad.reshape(NC, WPC, P).transpose(0, 2, 1)[:, :, :, None] == gids
    ).astype(FP8)                                # [c, p, w, 64]
    ohg_all = ohg_all.reshape(NC, P, WPC * D)
    g_counts = np.bincount(n2g_in, minlength=NG).astype(F32)
    invc_np = (1.0 / np.maximum(g_counts, 1.0)).reshape(NG, 1)

    W0 = np.asarray(W0, F32)
    common = {
        "w0r": np.tile(W0.reshape(1, D), (P, 1)).astype(BF16),
        "b0r": np.tile(np.asarray(b0, F32).reshape(1, D), (P, 1)).astype(BF16),
        "w1a": np.concatenate(
            [np.asarray(W1, F32), np.asarray(b1, F32).reshape(1, D)], axis=0
        ).astype(BF16),
        "w2a": np.concatenate(
            [np.asarray(W2, F32), np.asarray(b2, F32).reshape(1, D)], axis=0
        ).astype(BF16),
        "wr": np.asarray(Wr, F32).reshape(D, 1).astype(BF16),
        "invc": invc_np,
        "brc": np.full((NG, 1), float(np.asarray(br).reshape(-1)[0]), dtype=F32),
    }
    in_maps = []
    for c in range(NC):
        m = dict(common)
        m["ohd"] = oh_all[c]
        m["ohgd"] = np.ascontiguousarray(ohg_all[c])
        m["ga"] = ga_w[c]
        m["gb"] = gb_w[c]
        m["a0n"] = np.ascontiguousarray(a0n_all[c])
        m["ns_c"] = np.ascontiguousarray(ns_all[c])
        m["nd_c"] = np.ascontiguousarray(nd_all[c])
        m["sn_c"] = np.ascontiguousarray(sn_all[c])
        m["rnd_d"] = np.ascontiguousarray(rnd_all[c])
        in_maps.append(m)
    return (M_A, M_B), in_maps


def kernel(src, dst, node2graph, W0, b0, W1, b1, W2, b2, Wr, br):
    global LAST_RESULTS
    (M_A, M_B), in_maps = _prep_inputs(
        src, dst, node2graph, W0, b0, W1, b1, W2, b2, Wr, br
    )
    key = (M_A, M_B)
    if key not in _PROGRAM_CACHE:
        _PROGRAM_CACHE[key] = _build_program(M_A, M_B)
    nc = _PROGRAM_CACHE[key]
    res = run_bass_kernel_spmd(nc, in_maps, core_ids=list(range(NC)))
    LAST_RESULTS = res
    return np.asarray(res.results[0]["y"], dtype=np.float32)
